# revision 8
# baseline (speedup 1.0000x reference)
"""Distributed Bass kernel for nn_Attention (LN -> QKV -> MHA -> out-proj).

Sharding (8 cores, SPMD-uniform graph):
  - core i computes heads {2i, 2i+1} for BOTH batches (tensor-parallel on heads)
  - after attention, one 8-core AllToAll redistributes head-channels -> token
    slices; core i finishes the out-projection for global tokens
    [512*i, 512*(i+1)) (batch i//4, rows 512*(i%4)...)

Device pipeline per core:
  A. x (bf16) -> transposed load x^T; LN stats via bn_stats on normal-layout
     copy; stats transposed via PE; (x^T - mu)*rstd -> xn^T (bf16)
  B. QKV: qT/kT = W^T-slice @ xn^T (PSUM f32, +bias); v in [tok, ch] layout
     (+bias via K=1 ones matmul); qT/kT duplicated into both 64-partition
     halves (enables k-tile pairing via tile_position row groups)
  C. per (batch, head): S^T = kT.T@qT row-group-paired k-tiles; exp on ScalarE
     (scale=Dh^-0.5 folded, no max-subtraction: |scores| ~ few sigma);
     O^T = [v|1].T @ P^T accumulated over k -> row 64 = softmax denominator
  D. AllToAll (bf16): shard j = [2*64 O^T rows + 2 denom rows, 512 toks of
     core j]
  E. normalize with gathered denominators (reciprocal + multiply)
  F. y = xa @ W_out + b_out for my 512 tokens -> out
"""

import sys

sys.path.insert(0, "/opt/trn_rl_repo")

import numpy as np
import ml_dtypes

DIM = 1024
HEADS = 16
B = 2
N = 2048
Dh = 64
NCORES = 8
T = B * N  # 4096 global tokens
HPC = 2  # heads per core
CHC = HPC * Dh  # 128 channels per core
SCALE = Dh**-0.5
BF16 = ml_dtypes.bfloat16

_cache = {}


def _build():
    import concourse.bass as bass
    import concourse.tile as tile
    from concourse import bacc, mybir
    from concourse.masks import make_identity

    fp32 = mybir.dt.float32
    bf16 = mybir.dt.bfloat16

    nc = bacc.Bacc("TRN2", target_bir_lowering=False, debug=False, num_devices=NCORES)

    x_ext = nc.dram_tensor("x", [T, DIM], bf16, kind="ExternalInput")
    wq_ext = nc.dram_tensor("wq", [DIM, CHC], bf16, kind="ExternalInput")
    wk_ext = nc.dram_tensor("wk", [DIM, CHC], bf16, kind="ExternalInput")
    wv_ext = nc.dram_tensor("wv", [DIM, CHC], bf16, kind="ExternalInput")
    bqk_ext = nc.dram_tensor("bqk", [128, 2], fp32, kind="ExternalInput")
    bv_ext = nc.dram_tensor("bv", [1, CHC], bf16, kind="ExternalInput")
    wo_ext = nc.dram_tensor("wo", [DIM, DIM], bf16, kind="ExternalInput")
    bo_ext = nc.dram_tensor("bo", [1, DIM], fp32, kind="ExternalInput")
    out_ext = nc.dram_tensor("out", [512, DIM], fp32, kind="ExternalOutput")

    NT = T // 128  # 32 token tiles
    NC = DIM // 128  # 8 channel chunks

    with tile.TileContext(nc) as tc:
        with (
            tc.tile_pool(name="persist", bufs=1) as persist,
            tc.tile_pool(name="dram", bufs=1, space="DRAM") as dram,
        ):
            identity = persist.tile([128, 128], fp32, tag="identity")
            make_identity(nc, identity)
            eps_ap = persist.tile([128, 1], fp32, tag="eps")
            nc.vector.memset(eps_ap, 1e-5)
            ones_col = persist.tile([1, 128], bf16, tag="ones_col")
            nc.vector.memset(ones_col, 1.0)

            # weights
            wq_sb = persist.tile([128, NC, CHC], bf16, tag="wq")
            wk_sb = persist.tile([128, NC, CHC], bf16, tag="wk")
            wv_sb = persist.tile([128, NC, CHC], bf16, tag="wv")
            nc.sync.dma_start(out=wq_sb, in_=wq_ext.ap().rearrange("(c p) m -> p c m", p=128))
            nc.sync.dma_start(out=wk_sb, in_=wk_ext.ap().rearrange("(c p) m -> p c m", p=128))
            nc.sync.dma_start(out=wv_sb, in_=wv_ext.ap().rearrange("(c p) m -> p c m", p=128))
            wo_sb = persist.tile([128, NC, DIM], bf16, tag="wo")
            nc.sync.dma_start(out=wo_sb, in_=wo_ext.ap().rearrange("(c p) m -> p c m", p=128))
            bqk_sb = persist.tile([128, 2], fp32, tag="bqk")
            nc.sync.dma_start(out=bqk_sb, in_=bqk_ext.ap())
            bv_sb = persist.tile([1, CHC], bf16, tag="bv")
            nc.sync.dma_start(out=bv_sb, in_=bv_ext.ap())
            bo_sb = persist.tile([128, DIM], fp32, tag="bo")
            nc.sync.dma_start(out=bo_sb, in_=bo_ext.ap().to_broadcast((128, DIM)))

            # persistent activations
            qT2 = [persist.tile([128, T], bf16, tag=f"qT2_{h}", name=f"qT2_{h}") for h in range(HPC)]
            kT2 = [persist.tile([128, T], bf16, tag=f"kT2_{h}", name=f"kT2_{h}") for h in range(HPC)]
            # v plus a ones column per head: [tok, 32 tile, head, 64 v + 1 one + 7 pad]
            v_ext_t = persist.tile([128, NT, HPC, 72], bf16, tag="v_ext")
            nc.vector.memset(v_ext_t[:, :, :, 64:65], 1.0)
            # O^T + denominator row per (batch, head) unit
            OTn = [persist.tile([65, N], bf16, tag=f"OTn_{u}", name=f"OTn_{u}") for u in range(B * HPC)]

            # ---------------- Phase A: load + LN ----------------
            with (
                tc.tile_pool(name="xpool", bufs=4) as xpool,
                tc.tile_pool(name="stats", bufs=1) as stats_pool,
                tc.tile_pool(name="xnbig", bufs=1) as xnbig,
                tc.tile_pool(name="psA", bufs=4, space="PSUM") as psA,
            ):
                xT = xnbig.tile([128, NC, T], bf16, tag="xT")
                for c in range(NC):
                    nc.sync.dma_start_transpose(
                        xT[:, c, :], x_ext.ap()[:, c * 128 : (c + 1) * 128]
                    )

                stats_mu = stats_pool.tile([128, NT], fp32, tag="stats_mu")
                stats_var = stats_pool.tile([128, NT], fp32, tag="stats_var")
                for t in range(NT):
                    x_t = xpool.tile([128, DIM], bf16, tag="x_t")
                    nc.sync.dma_start(out=x_t, in_=x_ext.ap()[t * 128 : (t + 1) * 128, :])
                    st = xpool.tile([128, 2, 6], fp32, tag="bn_st")
                    nc.vector.bn_stats(out=st[:, 0, :], in_=x_t[:, 0:512])
                    nc.vector.bn_stats(out=st[:, 1, :], in_=x_t[:, 512:1024])
                    mv = xpool.tile([128, 2], fp32, tag="bn_mv")
                    nc.vector.bn_aggr(out=mv, in_=st)
                    nc.gpsimd.tensor_copy(out=stats_mu[:, t : t + 1], in_=mv[:, 0:1])
                    nc.gpsimd.tensor_copy(out=stats_var[:, t : t + 1], in_=mv[:, 1:2])
                # rstd = 1/sqrt(var + eps)
                stats_rstd = stats_pool.tile([128, NT], fp32, tag="stats_rstd")
                nc.scalar.activation(
                    out=stats_rstd, in_=stats_var,
                    func=mybir.ActivationFunctionType.Sqrt, bias=eps_ap, scale=1.0,
                )
                nc.vector.reciprocal(out=stats_rstd, in_=stats_rstd)
                # transpose stats to free-dim layout: [NT, 128]
                ps_mu = psA.tile([NT, 128], fp32, tag="ps_bc")
                nc.tensor.transpose(ps_mu, stats_mu, identity)
                ps_rstd = psA.tile([NT, 128], fp32, tag="ps_bc")
                nc.tensor.transpose(ps_rstd, stats_rstd, identity)
                muT_sb = stats_pool.tile([NT, 128], bf16, tag="muT_sb")
                rstdT_sb = stats_pool.tile([NT, 128], bf16, tag="rstdT_sb")
                nc.vector.tensor_copy(out=muT_sb, in_=ps_mu)
                nc.vector.tensor_copy(out=rstdT_sb, in_=ps_rstd)
                mu_row = stats_pool.tile([1, T], bf16, tag="mu_row")
                rstd_row = stats_pool.tile([1, T], bf16, tag="rstd_row")
                nc.sync.dma_start(
                    out=mu_row.rearrange("o (t p) -> o t p", t=NT), in_=muT_sb
                )
                nc.sync.dma_start(
                    out=rstd_row.rearrange("o (t p) -> o t p", t=NT), in_=rstdT_sb
                )

                # broadcast stats across partitions (K=1 matmuls) + normalize
                xnT = xT  # normalized in place
                for tc4 in range(T // 512):  # 512-token chunks
                    mu_b = psA.tile([128, 512], fp32, tag="ps_bc")
                    rstd_b = psA.tile([128, 512], fp32, tag="ps_bc")
                    for j in range(4):
                        t = tc4 * 4 + j
                        nc.tensor.matmul(
                            mu_b[:, j * 128 : (j + 1) * 128], ones_col,
                            mu_row[:, t * 128 : (t + 1) * 128],
                            start=True, stop=True,
                        )
                        nc.tensor.matmul(
                            rstd_b[:, j * 128 : (j + 1) * 128], ones_col,
                            rstd_row[:, t * 128 : (t + 1) * 128],
                            start=True, stop=True,
                        )
                    xs = xT[:, :, tc4 * 512 : (tc4 + 1) * 512]
                    nc.vector.tensor_tensor(
                        xs, xs, mu_b[:, None, :].to_broadcast((128, NC, 512)),
                        mybir.AluOpType.subtract,
                    )
                    nc.vector.tensor_tensor(
                        xs, xs, rstd_b[:, None, :].to_broadcast((128, NC, 512)),
                        mybir.AluOpType.mult,
                    )

                # ---------------- Phase B: QKV ----------------
                qT_t = xnbig.tile([128, T], bf16, tag="qT_t")
                kT_t = xnbig.tile([128, T], bf16, tag="kT_t")
                for (w_sb, dst, bcol) in ((wq_sb, qT_t, 0), (wk_sb, kT_t, 1)):
                    for tc4 in range(T // 512):
                        ps = psA.tile([128, 512], fp32, tag="ps_qkv")
                        for c in range(NC):
                            nc.tensor.matmul(
                                ps, w_sb[:, c, :], xnT[:, c, tc4 * 512 : (tc4 + 1) * 512],
                                start=(c == 0), stop=(c == NC - 1),
                            )
                        nc.vector.tensor_scalar(
                            out=dst[:, tc4 * 512 : (tc4 + 1) * 512], in0=ps,
                            scalar1=bqk_sb[:, bcol : bcol + 1], scalar2=None,
                            op0=mybir.AluOpType.add,
                        )
                # duplicate halves so row-group pairing can run 2 k-tiles at once
                for h in range(HPC):
                    src_q = qT_t[h * 64 : (h + 1) * 64, :]
                    src_k = kT_t[h * 64 : (h + 1) * 64, :]
                    nc.sync.dma_start(out=qT2[h][0:64, :], in_=src_q)
                    nc.sync.dma_start(out=qT2[h][64:128, :], in_=src_q)
                    nc.sync.dma_start(out=kT2[h][0:64, :], in_=src_k)
                    nc.sync.dma_start(out=kT2[h][64:128, :], in_=src_k)

                # v: [tok, ch] layout, bias via K=1 ones matmul
                for t in range(NT):
                    ps = psA.tile([128, CHC], fp32, tag="ps_qkv")
                    nc.tensor.matmul(ps, ones_col, bv_sb, start=True, stop=False)
                    for c in range(NC):
                        nc.tensor.matmul(
                            ps, xnT[:, c, t * 128 : (t + 1) * 128], wv_sb[:, c, :],
                            start=False, stop=(c == NC - 1),
                        )
                    nc.vector.tensor_copy(
                        out=v_ext_t[:, t, :, 0:64],
                        in_=ps.rearrange("p (h d) -> p h d", h=HPC),
                    )

            # ---------------- Phase C: attention ----------------
            with (
                tc.tile_pool(name="pt", bufs=3) as ptpool,
                tc.tile_pool(name="attn_sb", bufs=2) as attn_sb,
                tc.tile_pool(name="psS", bufs=1, space="PSUM") as psS,
                tc.tile_pool(name="psO", bufs=2, space="PSUM") as psO,
            ):
                for u in range(B * HPC):
                    bt, h = u // HPC, u % HPC
                    tok0 = bt * N
                    kt0 = bt * (N // 128)  # first v tile of this batch
                    for qh in range(2):  # 1024-token halves of this batch
                        q0 = tok0 + qh * 1024
                        ps_o = psO.tile([128, 1024], fp32, tag="ps_o")
                        for kp in range(8):  # pairs of k-tiles
                            ps_s = psS.tile([128, 2, 1024], fp32, tag="ps_s")
                            for d in range(2):
                                kt = 2 * kp + d
                                lo, hi = d * 64, d * 64 + 64
                                for qc in range(2):
                                    nc.tensor.matmul(
                                        ps_s[:, d, qc * 512 : (qc + 1) * 512],
                                        kT2[h][lo:hi, tok0 + kt * 128 : tok0 + (kt + 1) * 128],
                                        qT2[h][lo:hi, q0 + qc * 512 : q0 + (qc + 1) * 512],
                                        start=True, stop=True,
                                        tile_position=(d * 64, 0),
                                    )
                            pt = ptpool.tile([128, 2, 1024], bf16, tag="pt")
                            nc.scalar.activation(
                                out=pt, in_=ps_s,
                                func=mybir.ActivationFunctionType.Exp, scale=SCALE,
                            )
                            for d in range(2):
                                kt = 2 * kp + d
                                for qc in range(2):
                                    nc.tensor.matmul(
                                        ps_o[0:65, qc * 512 : (qc + 1) * 512],
                                        v_ext_t[:, kt0 + kt, h, 0:65],
                                        pt[:, d, qc * 512 : (qc + 1) * 512],
                                        start=(kp == 0 and d == 0),
                                        stop=(kp == 7 and d == 1),
                                    )
                        nc.vector.tensor_copy(
                            out=OTn[u][:, qh * 1024 : (qh + 1) * 1024], in_=ps_o[0:65, :]
                        )

            # ---------------- Phase D/E/F: A2A, normalize, out-proj ----------------
            with (
                tc.tile_pool(name="fin", bufs=1) as fin,
                tc.tile_pool(name="psY", bufs=4, space="PSUM") as psY,
            ):
                in_b = dram.tile([NCORES * 130, 512], bf16)
                out_b = dram.tile([NCORES * 130, 512], bf16)
                for j in range(NCORES):
                    u = (j // 4) * HPC  # unit base for batch j//4
                    col0 = (j % 4) * 512
                    for h in range(HPC):
                        nc.sync.dma_start(
                            out=in_b[j * 130 + h * 64 : j * 130 + (h + 1) * 64, :],
                            in_=OTn[u + h][0:64, col0 : col0 + 512],
                        )
                        nc.sync.dma_start(
                            out=in_b[j * 130 + 128 + h : j * 130 + 129 + h, :],
                            in_=OTn[u + h][64:65, col0 : col0 + 512],
                        )
                nc.gpsimd.collective_compute(
                    "AllToAll",
                    mybir.AluOpType.bypass,
                    replica_groups=[list(range(NCORES))],
                    ins=[in_b.opt()],
                    outs=[out_b.opt()],
                )
                xa_raw = fin.tile([128, NC, 512], bf16, tag="xa_raw")
                dnm = fin.tile([128, NC, 512], bf16, tag="dnm")
                for c in range(NC):
                    nc.sync.dma_start(
                        out=xa_raw[:, c, :], in_=out_b[c * 130 : c * 130 + 128, :]
                    )
                    nc.sync.dma_start(
                        out=dnm[0:64, c, :],
                        in_=out_b[c * 130 + 128 : c * 130 + 129, :].to_broadcast((64, 512)),
                    )
                    nc.sync.dma_start(
                        out=dnm[64:128, c, :],
                        in_=out_b[c * 130 + 129 : c * 130 + 130, :].to_broadcast((64, 512)),
                    )
                rcp = fin.tile([128, NC, 512], fp32, tag="rcp")
                nc.vector.reciprocal(out=rcp, in_=dnm)
                xa = fin.tile([128, NC, 512], bf16, tag="xa")
                nc.vector.tensor_tensor(xa, xa_raw, rcp, mybir.AluOpType.mult)

                for mt in range(4):  # my 512 tokens in 128-tiles
                    y = fin.tile([128, DIM], fp32, tag="y")
                    for nh in range(2):
                        ps = psY.tile([128, 512], fp32, tag="ps_y")
                        for c in range(NC):
                            nc.tensor.matmul(
                                ps,
                                xa[:, c, mt * 128 : (mt + 1) * 128],
                                wo_sb[:, c, nh * 512 : (nh + 1) * 512],
                                start=(c == 0), stop=(c == NC - 1),
                            )
                        nc.vector.tensor_tensor(
                            y[:, nh * 512 : (nh + 1) * 512], ps,
                            bo_sb[:, nh * 512 : (nh + 1) * 512], mybir.AluOpType.add,
                        )
                    nc.sync.dma_start(
                        out=out_ext.ap()[mt * 128 : (mt + 1) * 128, :], in_=y
                    )

    nc.compile()
    return nc


def _prep_inputs(x, ln_gamma, ln_beta, W_qkv, W_out, b_out):
    """Host-side: fold gamma/beta into W_qkv, slice per core, cast to bf16."""
    Wf = ln_gamma[:, None].astype(np.float64) * W_qkv.astype(np.float64)
    bf = ln_beta.astype(np.float64) @ W_qkv.astype(np.float64)  # [3*DIM]
    x_all = x.reshape(T, DIM).astype(BF16)
    wo = W_out.astype(BF16)
    bo = b_out.astype(np.float32).reshape(1, DIM)
    in_maps = []
    for i in range(NCORES):
        c0 = i * CHC  # channel block of this core's 2 heads
        wq = Wf[:, 0 * DIM + c0 : 0 * DIM + c0 + CHC]
        wk = Wf[:, 1 * DIM + c0 : 1 * DIM + c0 + CHC]
        wv = Wf[:, 2 * DIM + c0 : 2 * DIM + c0 + CHC]
        bq = bf[0 * DIM + c0 : 0 * DIM + c0 + CHC]
        bk = bf[1 * DIM + c0 : 1 * DIM + c0 + CHC]
        bv = bf[2 * DIM + c0 : 2 * DIM + c0 + CHC]
        bqk = np.stack([bq, bk], axis=1).astype(np.float32)  # [128, 2]
        in_maps.append(
            {
                "x": x_all,
                "wq": np.ascontiguousarray(wq.astype(BF16)),
                "wk": np.ascontiguousarray(wk.astype(BF16)),
                "wv": np.ascontiguousarray(wv.astype(BF16)),
                "bqk": np.ascontiguousarray(bqk),
                "bv": np.ascontiguousarray(bv.astype(BF16).reshape(1, CHC)),
                "wo": wo,
                "bo": bo,
            }
        )
    return in_maps


def kernel(x, ln_gamma, ln_beta, W_qkv, W_out, b_out, _want_time=False):
    x = np.asarray(x, dtype=np.float32)
    ln_gamma = np.asarray(ln_gamma, dtype=np.float32)
    ln_beta = np.asarray(ln_beta, dtype=np.float32)
    W_qkv = np.asarray(W_qkv, dtype=np.float32)
    W_out = np.asarray(W_out, dtype=np.float32)
    b_out = np.asarray(b_out, dtype=np.float32)

    if "nc" not in _cache:
        _cache["nc"] = _build()
    nc = _cache["nc"]

    from concourse.bass_utils import run_bass_kernel_spmd

    in_maps = _prep_inputs(x, ln_gamma, ln_beta, W_qkv, W_out, b_out)
    res = run_bass_kernel_spmd(
        nc, in_maps, core_ids=list(range(NCORES)), trace=_want_time
    )
    out = np.empty((B, N, DIM), dtype=np.float32)
    for i in range(NCORES):
        b, g = i // 4, i % 4
        out[b, g * 512 : (g + 1) * 512, :] = res.results[i]["out"]
    if _want_time:
        return out, res.exec_time_ns
    return out
